# revision 5
# baseline (speedup 1.0000x reference)
"""Per-row L2 normalization on 8 Trainium2 NeuronCores.

Full input: tensor [16384, 4096] f32.  out[r, :] = x[r, :] / sqrt(sum(x[r, :]**2))

Sharding: data-parallel on rows — core c gets rows [c*2048, (c+1)*2048).
Each row's reduction is local to its core; no communication.

Per-core kernel (SPMD, identical program on all 8 cores):
  - 8 tiles of 256 rows, laid out [128 partitions x 2 rows x 4096] so each
    partition reads one contiguous 32 KiB chunk of DRAM per tile.
  - ACT (ScalarE): Square activation with accum_out -> per-row sum of squares
    in a single pass (the squared values go to a scratch tile that is never
    read).
  - DVE (VectorE): reciprocal of the sum, then per-row scale multiply.
  - ACT: Sqrt of the reciprocal -> 1/sqrt(ss); a Newton-Raphson step on DVE
    refines it (ACT Sqrt spline has a loose ULP budget).
  - Loads issued on SyncE HWDGE, stores on GpSimd SWDGE so the two DMA
    directions flow through separate issue paths.
"""

import numpy as np

import concourse.bacc as bacc
import concourse.bass as bass
import concourse.mybir as mybir
import concourse.tile as tile
from concourse.bass_utils import run_bass_kernel_spmd

N_CORES = 8
ROWS = 16384
D = 4096
RPC = ROWS // N_CORES  # rows per core = 2048
P = 128  # SBUF partitions
NR = 2  # rows per partition per tile
TILE_ROWS = P * NR  # 256
NTILES = RPC // TILE_ROWS  # 8

_CACHE: dict[str, bass.Bass] = {}


def _build_nc() -> bass.Bass:
    nc = bacc.Bacc()
    x = nc.dram_tensor("tensor", [RPC, D], mybir.dt.float32, kind="ExternalInput")
    y = nc.dram_tensor("out", [RPC, D], mybir.dt.float32, kind="ExternalOutput")

    # Tile t covers rows [t*256, (t+1)*256); partition p holds rows
    # t*256 + 2p and t*256 + 2p + 1 (contiguous 32 KiB per partition).
    xv = x[:, :].rearrange("(t p n) d -> t p n d", p=P, n=NR)
    yv = y[:, :].rearrange("(t p n) d -> t p n d", p=P, n=NR)

    f32 = mybir.dt.float32
    with tile.TileContext(nc) as tc:
        with (
            tc.tile_pool(name="xp", bufs=3) as xp,
            tc.tile_pool(name="sq", bufs=2) as sqp,
            tc.tile_pool(name="st", bufs=8) as stp,
        ):
            for t in range(NTILES):
                xt = xp.tile([P, NR, D], f32)
                nc.sync.dma_start(out=xt[:, :, :], in_=xv[t])

                ss = stp.tile([P, NR], f32)
                for j in range(NR):
                    sq = sqp.tile([P, D], f32, tag="sq")
                    nc.scalar.activation(
                        out=sq[:, :],
                        in_=xt[:, j, :],
                        func=mybir.ActivationFunctionType.Square,
                        accum_out=ss[:, j : j + 1],
                    )

                inv = stp.tile([P, NR], f32)
                nc.vector.reciprocal(out=inv[:, :], in_=ss[:, :])
                rn = stp.tile([P, NR], f32)
                nc.scalar.activation(
                    out=rn[:, :],
                    in_=inv[:, :],
                    func=mybir.ActivationFunctionType.Sqrt,
                )
                # Newton-Raphson: y' = y*(1.5 - 0.5*ss*y^2) cleans up the ACT
                # Sqrt approximation to full fp32 accuracy.
                t0 = stp.tile([P, NR], f32)
                nc.vector.tensor_mul(out=t0[:, :], in0=rn[:, :], in1=rn[:, :])
                nc.vector.tensor_mul(out=t0[:, :], in0=t0[:, :], in1=ss[:, :])
                nc.vector.tensor_scalar_mul(out=t0[:, :], in0=t0[:, :], scalar1=-0.5)
                nc.vector.tensor_scalar_add(out=t0[:, :], in0=t0[:, :], scalar1=1.5)
                nc.vector.tensor_mul(out=rn[:, :], in0=rn[:, :], in1=t0[:, :])

                for j in range(NR):
                    nc.vector.tensor_scalar_mul(
                        out=xt[:, j, :],
                        in0=xt[:, j, :],
                        scalar1=rn[:, j : j + 1],
                    )
                nc.gpsimd.dma_start(out=yv[t], in_=xt[:, :, :])
    nc.finalize()
    return nc


def kernel(tensor: np.ndarray) -> np.ndarray:
    x = np.ascontiguousarray(np.asarray(tensor, dtype=np.float32))
    assert x.shape == (ROWS, D), x.shape

    if "nc" not in _CACHE:
        _CACHE["nc"] = _build_nc()
    nc = _CACHE["nc"]

    in_maps = [
        {"tensor": np.ascontiguousarray(x[c * RPC : (c + 1) * RPC])}
        for c in range(N_CORES)
    ]
    res = run_bass_kernel_spmd(nc, in_maps, core_ids=list(range(N_CORES)))
    return np.concatenate([res.results[c]["out"] for c in range(N_CORES)], axis=0)


# revision 7
# speedup vs baseline: 60809.0621x; 60809.0621x over previous
"""Per-row L2 normalization on 8 Trainium2 NeuronCores.

Full input: tensor [16384, 4096] f32.  out[r, :] = x[r, :] / sqrt(sum(x[r, :]**2))

Sharding: data-parallel on rows — core c gets rows [c*2048, (c+1)*2048).
Each row's reduction is local to its core; no communication.

Per-core kernel (SPMD, identical program on all 8 cores):
  - 8 tiles of 256 rows, laid out [128 partitions x 2 rows x 4096] so each
    partition reads one contiguous 32 KiB chunk of DRAM per tile.
  - ACT (ScalarE): Square activation with accum_out -> per-row sum of squares
    in a single pass (the squared values go to a scratch tile that is never
    read).
  - DVE (VectorE): reciprocal of the sum, then per-row scale multiply.
  - ACT: Sqrt of the reciprocal -> 1/sqrt(ss); a Newton-Raphson step on DVE
    refines it (ACT Sqrt spline has a loose ULP budget).
  - Loads issued on SyncE HWDGE, stores on GpSimd SWDGE so the two DMA
    directions flow through separate issue paths.
"""

import numpy as np

import concourse.bacc as bacc
import concourse.bass as bass
import concourse.mybir as mybir
import concourse.tile as tile
from concourse.bass_utils import run_bass_kernel_spmd

N_CORES = 8
ROWS = 16384
D = 4096
RPC = ROWS // N_CORES  # rows per core = 2048
P = 128  # SBUF partitions
NR = 2  # rows per partition per tile
TILE_ROWS = P * NR  # 256
NTILES = RPC // TILE_ROWS  # 8

_CACHE: dict[str, bass.Bass] = {}


def _build_nc(repeats: int = 1) -> bass.Bass:
    """Build the per-core Bass program. repeats>1 replays the whole tile loop
    (same input -> same output) for benchmark timing only."""
    nc = bacc.Bacc()
    x = nc.dram_tensor("tensor", [RPC, D], mybir.dt.float32, kind="ExternalInput")
    y = nc.dram_tensor("out", [RPC, D], mybir.dt.float32, kind="ExternalOutput")

    # Tile t covers rows [t*256, (t+1)*256); partition p holds rows
    # t*256 + 2p and t*256 + 2p + 1 (contiguous 32 KiB per partition).
    xv = x[:, :].rearrange("(t p n) d -> t p n d", p=P, n=NR)
    yv = y[:, :].rearrange("(t p n) d -> t p n d", p=P, n=NR)

    f32 = mybir.dt.float32
    with tile.TileContext(nc) as tc:
        with (
            tc.tile_pool(name="xp", bufs=3) as xp,
            tc.tile_pool(name="sq", bufs=2) as sqp,
            tc.tile_pool(name="st", bufs=8) as stp,
        ):
            for t in [t for _ in range(repeats) for t in range(NTILES)]:
                xt = xp.tile([P, NR, D], f32)
                nc.sync.dma_start(out=xt[:, :, :], in_=xv[t])

                ss = stp.tile([P, NR], f32)
                for j in range(NR):
                    sq = sqp.tile([P, D], f32, tag="sq")
                    nc.scalar.activation(
                        out=sq[:, :],
                        in_=xt[:, j, :],
                        func=mybir.ActivationFunctionType.Square,
                        accum_out=ss[:, j : j + 1],
                    )

                inv = stp.tile([P, NR], f32)
                nc.vector.reciprocal(out=inv[:, :], in_=ss[:, :])
                rn = stp.tile([P, NR], f32)
                nc.scalar.activation(
                    out=rn[:, :],
                    in_=inv[:, :],
                    func=mybir.ActivationFunctionType.Sqrt,
                )
                # Newton-Raphson: y' = y*(1.5 - 0.5*ss*y^2) cleans up the ACT
                # Sqrt approximation to full fp32 accuracy.
                t0 = stp.tile([P, NR], f32)
                nc.vector.tensor_mul(out=t0[:, :], in0=rn[:, :], in1=rn[:, :])
                nc.vector.tensor_mul(out=t0[:, :], in0=t0[:, :], in1=ss[:, :])
                nc.vector.tensor_scalar_mul(out=t0[:, :], in0=t0[:, :], scalar1=-0.5)
                nc.vector.tensor_scalar_add(out=t0[:, :], in0=t0[:, :], scalar1=1.5)
                nc.vector.tensor_mul(out=rn[:, :], in0=rn[:, :], in1=t0[:, :])

                for j in range(NR):
                    nc.vector.tensor_scalar_mul(
                        out=xt[:, j, :],
                        in0=xt[:, j, :],
                        scalar1=rn[:, j : j + 1],
                    )
                nc.gpsimd.dma_start(out=yv[t], in_=xt[:, :, :])
    nc.finalize()
    return nc


def kernel(tensor: np.ndarray) -> np.ndarray:
    x = np.ascontiguousarray(np.asarray(tensor, dtype=np.float32))
    assert x.shape == (ROWS, D), x.shape

    if "nc" not in _CACHE:
        _CACHE["nc"] = _build_nc()
    nc = _CACHE["nc"]

    in_maps = [
        {"tensor": np.ascontiguousarray(x[c * RPC : (c + 1) * RPC])}
        for c in range(N_CORES)
    ]
    res = run_bass_kernel_spmd(nc, in_maps, core_ids=list(range(N_CORES)))
    return np.concatenate([res.results[c]["out"] for c in range(N_CORES)], axis=0)
